# revision 2
# baseline (speedup 1.0000x reference)
"""Trainium2 Bass kernel for NeighborhoodNormalization.

Math: the reference builds a per-point homogeneous transform
T = [[ux,-uy,0,px],[uy,ux,0,py],[0,0,1,pz],[0,0,0,1]] (u = p/||p||),
inverts it, and applies it to 64 neighbors per point.  With
r2 = px^2+py^2, n = ||p||, a = n/r2, cx = px*a, cy = py*a and
d = q - p (per-neighbor delta), the output reduces to

    out.x =  cx*dx + cy*dy
    out.y = -cy*dx + cx*dy
    out.z =  dz

Pure elementwise math (memory-bound; ~25.4 MB of HBM traffic per core at
a ~430 GB/s shared in+out DMA cap -> ~59 us floor).  Sharding: pure data
parallel over the N=8192 point axis across 8 cores (1024 points/core).

Per-core layout: 16384 points = 128 partitions x 128 columns, partition
p = b*8 + s holds points with local n = s*128 + t (t = column).  Neighbor
rows (64*3 floats) are contiguous in HBM per point, so DMAs move
[128 partitions x 12 KiB contiguous] blocks (16 columns per group).

Compute per group of 16 columns (wide 4D tensor_tensor ops with 0-stride
broadcast of the per-column coefficients along K):
    SUB:  ot = nb - p          (V; z-component via ACT per-column on 6/8
                                groups to offload the vector engine)
    PCx:  Q[0:2] = d_xy * (cx, cy)    (V)
    PDy:  Q[2:4] = d_xy * (-cy, cx)   (GPS)
    SXY:  ot_xy = Q[even2] + Q[odd2]  (split half V / half GPS)
Engine busy ~51 us V / ~49 us GPS / ~42 us ACT, all under the DMA floor.
"""

import sys

if "/opt/trn_rl_repo" not in sys.path:
    sys.path.insert(0, "/opt/trn_rl_repo")

import numpy as np

import concourse.bass as bass
import concourse.bacc as bacc
import concourse.mybir as mybir
from concourse.tile import TileContext
from concourse.bass_utils import run_bass_kernel_spmd

B = 16
N = 8192
K = 64
NCORES = 8
NLOC = N // NCORES  # 1024 points per core
P = 128             # SBUF partitions
S = NLOC // P       # 8 partition sub-blocks per batch entry
T = (B * NLOC) // P  # 128 point-columns per partition
G = 16              # columns per DMA group
NG = T // G
G2 = G // 2

F32 = mybir.dt.float32
OP = mybir.AluOpType
AF = mybir.ActivationFunctionType

# groups whose z-subtraction runs per-column on the ACT engine
ACT_Z_GROUPS = frozenset(range(6))

_CACHE = {}


def _build_nc():
    nc = bacc.Bacc(None, target_bir_lowering=False)

    pts = nc.declare_dram_parameter("points", [B, NLOC, 3], F32, isOutput=False)
    nb = nc.declare_dram_parameter("neighborhoods", [B, NLOC, K, 3], F32, isOutput=False)
    out = nc.declare_dram_parameter("out", [B, NLOC, K, 3], F32, isOutput=True)

    # partition = (b s), columns = t, free = 192 floats per point
    nbr = nb[:].rearrange("b (s t) k c -> (b s) t (k c)", s=S)
    outr = out[:].rearrange("b (s t) k c -> (b s) t (k c)", s=S)
    ptsr = pts[:].rearrange("b (s t) c -> (b s) (t c)", s=S)

    with TileContext(nc) as tc:
        with tc.tile_pool(name="const", bufs=1) as cpool, \
             tc.tile_pool(name="io_in", bufs=4) as inpool, \
             tc.tile_pool(name="io_out", bufs=4) as outpool, \
             tc.tile_pool(name="quad", bufs=3) as qpool:

            pts_sb = cpool.tile([P, T, 3], F32, tag="pts")
            nc.sync.dma_start(
                out=pts_sb[:].rearrange("p t c -> p (t c)"), in_=ptsr)
            px = pts_sb[:, :, 0]
            py = pts_sb[:, :, 1]
            pz = pts_sb[:, :, 2]

            def ctile(tag, w=T):
                return cpool.tile([P, w], F32, tag=tag, name=tag)

            t1 = ctile("t1")
            t2 = ctile("t2")
            r2 = ctile("r2")
            n2 = ctile("n2")
            nn = ctile("nn")
            ir2 = ctile("ir2")
            aa = ctile("aa")
            npz = ctile("npz")
            cA = cpool.tile([P, T, 2], F32, tag="cA", name="cA")  # (cx,  cy)
            cD = cpool.tile([P, T, 2], F32, tag="cD", name="cD")  # (-cy, cx)

            nc.vector.tensor_mul(out=t1[:], in0=px, in1=px)
            nc.vector.tensor_mul(out=t2[:], in0=py, in1=py)
            nc.vector.tensor_add(out=r2[:], in0=t1[:], in1=t2[:])
            nc.vector.tensor_mul(out=t1[:], in0=pz, in1=pz)
            nc.vector.tensor_add(out=n2[:], in0=r2[:], in1=t1[:])
            nc.scalar.sqrt(out=nn[:], in_=n2[:])
            nc.vector.reciprocal(out=ir2[:], in_=r2[:])
            nc.vector.tensor_mul(out=aa[:], in0=nn[:], in1=ir2[:])
            # cA = (cx, cy) = (px*a, py*a); cD = (-cy, cx)
            nc.vector.tensor_mul(out=cA[:, :, 0], in0=px, in1=aa[:])
            nc.vector.tensor_mul(out=cA[:, :, 1], in0=py, in1=aa[:])
            nc.vector.scalar_tensor_tensor(
                out=cD[:, :, 0], in0=py, scalar=-1.0, in1=aa[:],
                op0=OP.mult, op1=OP.mult)
            nc.vector.tensor_mul(out=cD[:, :, 1], in0=px, in1=aa[:])
            nc.gpsimd.tensor_scalar(
                out=npz[:], in0=pz, scalar1=-1.0, scalar2=None, op0=OP.mult)

            for g in range(NG):
                gs, ge = g * G, (g + 1) * G
                nb_t = inpool.tile([P, G, K, 3], F32, tag="nb", name=f"nb{g}")
                nc.sync.dma_start(
                    out=nb_t[:].rearrange("p g k c -> p g (k c)"),
                    in_=nbr[:, gs:ge, :],
                )
                ot = outpool.tile([P, G, K, 3], F32, tag="ot", name=f"ot{g}")
                Q = qpool.tile([P, G, K, 4], F32, tag="Q", name=f"Q{g}")
                Q5 = Q[:].rearrange("p g k (two c) -> p g k two c", two=2)

                ot_xy = ot[:, :, :, 0:2]
                bp_xy = pts_sb[:, gs:ge, None, 0:2].broadcast_to([P, G, K, 2])
                bp3 = pts_sb[:, gs:ge, None, :].broadcast_to([P, G, K, 3])
                bcA = cA[:, gs:ge, None, :].broadcast_to([P, G, K, 2])
                bcD = cD[:, gs:ge, None, :].broadcast_to([P, G, K, 2])

                if g in ACT_Z_GROUPS:
                    # d_xy wide on V; z per-column on ACT
                    nc.vector.tensor_sub(out=ot_xy, in0=nb_t[:, :, :, 0:2],
                                         in1=bp_xy)
                    for i in range(G):
                        t = gs + i
                        nc.scalar.activation(
                            out=ot[:, i, :, 2], in_=nb_t[:, i, :, 2],
                            func=AF.Identity, bias=npz[:, t:t + 1], scale=1.0)
                else:
                    # full 3-component subtract on V
                    nc.vector.tensor_sub(out=ot[:], in0=nb_t[:], in1=bp3)

                # Q[0:2] = (cx*dx, cy*dy) on V
                nc.vector.tensor_mul(out=Q[:, :, :, 0:2], in0=ot_xy, in1=bcA)
                # Q[2:4] = (-cy*dx, cx*dy) on GPS
                nc.gpsimd.tensor_mul(out=Q[:, :, :, 2:4], in0=ot_xy, in1=bcD)
                # ot_xy = Q[even2] + Q[odd2]; halves split V / GPS
                nc.vector.tensor_add(
                    out=ot_xy[:, 0:G2], in0=Q5[:, 0:G2, :, :, 0],
                    in1=Q5[:, 0:G2, :, :, 1])
                nc.gpsimd.tensor_add(
                    out=ot_xy[:, G2:G], in0=Q5[:, G2:G, :, :, 0],
                    in1=Q5[:, G2:G, :, :, 1])

                # out-DMA on the ACT HWDGE ring so it overlaps the SP-ring
                # input stream (HWDGE is FIFO per issuing engine).
                nc.scalar.dma_start(
                    out=outr[:, gs:ge, :],
                    in_=ot[:].rearrange("p g k c -> p g (k c)"),
                )

    nc.compile()
    return nc


def _get_nc():
    if "nc" not in _CACHE:
        _CACHE["nc"] = _build_nc()
    return _CACHE["nc"]


def kernel(points, neighborhoods):
    pts = np.ascontiguousarray(np.asarray(points, dtype=np.float32))
    nb = np.ascontiguousarray(np.asarray(neighborhoods, dtype=np.float32))
    assert pts.shape == (B, N, 3), pts.shape
    assert nb.shape == (B, N, K, 3), nb.shape

    in_maps = []
    for c in range(NCORES):
        sl = slice(c * NLOC, (c + 1) * NLOC)
        in_maps.append({
            "points": np.ascontiguousarray(pts[:, sl]),
            "neighborhoods": np.ascontiguousarray(nb[:, sl]),
        })

    res = run_bass_kernel_spmd(_get_nc(), in_maps, list(range(NCORES))).results
    out = np.concatenate([res[c]["out"] for c in range(NCORES)], axis=1)
    return out
